# revision 34
# baseline (speedup 1.0000x reference)
"""Trainium2 Bass kernel for nn_Attention_6983616824059.

Single-head attention, B=8, S=2048, H=256, K=32:
    q = x@Wq + bq ; k = x@Wk (+bk cancels in softmax) ; v = x@Wv + bv
    out = gamma * softmax(q k^T) v + x

Sharding: data-parallel over batch, 1 batch element per NeuronCore (8 cores).

Per-core algorithm (PE-facing data bf16, accumulation fp32):
  - load x [2048,256], PE-transpose to xT [256,2048] bf16
  - [qT;kT] = [Wq|Wk]^T xT  (one packed matmul), qT += bq
  - v = x Wv + bv [2048,256] + ones column (gives softmax denom for free)
  - scoresT[j,i] = kT_chunk^T qT : K=32 contraction, 4 j-chunks packed into
    the 128x128 PE array via tile_position row groups (qT replicated to all
    4 partition groups, kT chunks regrouped into kTp)
  - expT = exp(scoresT) (ScalarE, PSUM->SBUF, bf16)
  - out_unnorm = sum_j expT_chunk^T @ v_chunk (PSUM accumulation)
  - y = (gamma / D) * out_unnorm[:, :256] + x
"""

import sys
import numpy as np

sys.path.insert(0, "/opt/trn_rl_repo")

import ml_dtypes  # noqa: E402
import concourse.bass as bass  # noqa: E402
import concourse.tile as tile  # noqa: E402
from concourse import bacc, mybir  # noqa: E402
from concourse.bass_utils import run_bass_kernel_spmd  # noqa: E402

P = 128          # partitions
S = 2048         # sequence
H = 256          # hidden
KD = 32          # q/k head dim
SC = S // P      # 16 s-chunks (j-chunks)
HH = H // P      # 2 h-chunks
IW = 256         # i-slice width per pass
NPASS = S // IW  # 8 passes
ICP = IW // P    # 2 i-chunks per pass
NQ = SC // 4     # 4 j-quads per pass
VN = H + 2       # v free width: 256 + ones col + pad (col 257 = dup ones)

F32 = mybir.dt.float32
BF16 = mybir.dt.bfloat16
AF = mybir.ActivationFunctionType
ALU = mybir.AluOpType

import os
# 0 = no packing, 1 = packed + two outputs per PSUM bank,
# 2 = packed + one output per PSUM bank (strided exp read),
# 3 = packed + one output per bank, two 2-bank tiles per quad (pipelined)
PACK_MODE = int(os.environ.get("PACK_MODE", "3"))
PACK_SCORES = PACK_MODE > 0
# 0 = PE transpose of f32 x, 1 = DMA xbar transpose of host-cast bf16 x
TR_MODE = int(os.environ.get("TR_MODE", "1"))
# 1 = scale-step of the normalization on ScalarE + y stores on the ACT
# HWDGE ring (splits the pass-end latency chain across engines)
NORM_MODE = int(os.environ.get("NORM_MODE", "0"))
# 1 = fp8e5 DoubleRow attn@v: exp output and v in fp8e5, two j-chunks
# contracted per matmul (128x256 virtual array). exp biased by -2 for
# range margin (softmax-invariant). Needs VN padded so the chunk stride
# is a multiple of 16 bytes.
DR_MODE = int(os.environ.get("DR_MODE", "0"))
VNP = 272 if DR_MODE else VN
EDT = None  # set below
FP8 = mybir.dt.float8e5


def emit_body(nc, tc, d):
    x_d, wqk_d, wv_d, bq_d, bvb_d, gmb_d, idn_d, y_d = (
        d["x"], d["wqk"], d["wv"], d["bq"], d["bvb"], d["gmb"], d["idn"],
        d["y"])
    xb_d = d["xb"]

    with tc.tile_pool(name="const", bufs=1) as const, \
         tc.tile_pool(name="big", bufs=1) as big:
        # --- constants ---
        wqk_sb = const.tile([P, HH * 2 * KD], BF16)  # h-chunk hh at [:, hh*64:]
        wv_sb = const.tile([P, HH * H], BF16)        # h-chunk hh at [:, hh*H:]
        bq_sb = const.tile([KD, 1], F32)
        bvb_sb = const.tile([P, H], F32)
        gmb_sb = const.tile([P, 1], F32)
        idn_sb = const.tile([P, P], F32)
        for hh in range(HH):
            nc.sync.dma_start(wqk_sb[:, hh * 2 * KD:(hh + 1) * 2 * KD],
                              wqk_d[hh * P:(hh + 1) * P, :])
            nc.sync.dma_start(wv_sb[:, hh * H:(hh + 1) * H],
                              wv_d[hh * P:(hh + 1) * P, :])
        ebias_sb = const.tile([P, 1], F32)
        nc.gpsimd.memset(ebias_sb[:], -2.0)
        nc.sync.dma_start(bq_sb[:], bq_d[:])
        nc.sync.dma_start(bvb_sb[:], bvb_d[:])
        nc.sync.dma_start(gmb_sb[:], gmb_d[:])
        nc.sync.dma_start(idn_sb[:], idn_d[:])

        # --- resident tensors ---
        xall = big.tile([P, SC * H], F32)      # s-chunk sc at [:, sc*H:]
        xT = big.tile([P, HH * S], BF16)       # h-chunk hh at [:, hh*S + s]
        qT4 = big.tile([P, S], BF16)           # qT replicated to 4 part-groups
        kTn = big.tile([2 * KD, S], BF16)      # kT in rows 32:64
        kTp = big.tile([P, NQ * P], BF16)      # kTp[32m+p, t*128+c] = chunk 4t+m
        vall = big.tile([P, SC * VNP], FP8 if DR_MODE else BF16)

        # --- load x (4 batched DMAs) ---
        for g in range(4):
            src = x_d[g * 4 * P:(g + 1) * 4 * P, :].rearrange(
                "(q p) h -> p q h", p=P)
            dst = xall[:, g * 4 * H:(g + 1) * 4 * H].rearrange(
                "p (q h) -> p q h", q=4)
            nc.sync.dma_start(dst, src)

        with tc.tile_pool(name="sps", bufs=4, space="PSUM") as sps:
            xT3 = xT.rearrange("p (hh s) -> p hh s", hh=HH)
            if TR_MODE == 1:
                # xbar DMA transpose straight from DRAM (bf16 copy of x)
                xb3 = xb_d.rearrange("s (hh c) -> s hh c", c=P)
                for hh in range(HH):
                    nc.sync.dma_start(out=xT3[:, hh], in_=xb3[:, hh],
                                      transpose=True)
            else:
                # PE transpose of f32 x; PSUM->SBUF cast copies on DVE/ACT
                for sc in range(SC):
                    tr = sps.tile([P, H], F32, tag="ps")
                    for hh in range(HH):
                        nc.tensor.transpose(
                            tr[:, hh * P:(hh + 1) * P],
                            xall[:, sc * H + hh * P: sc * H + (hh + 1) * P],
                            idn_sb[:])
                    eng = nc.vector.tensor_copy if sc % 2 == 0 else (
                        lambda o, i: nc.scalar.activation(o, i, AF.Copy))
                    eng(xT3[:, :, sc * P:(sc + 1) * P],
                        tr.rearrange("p (hh c) -> p hh c", hh=HH))

            # --- qT / kT (one packed matmul per 512-slice) ---
            for i4 in range(4):
                qkps = sps.tile([2 * KD, 512], F32, tag="ps")
                for hh in range(HH):
                    nc.tensor.matmul(
                        qkps[:], wqk_sb[:, hh * 2 * KD:(hh + 1) * 2 * KD],
                        xT[:, hh * S + i4 * 512: hh * S + (i4 + 1) * 512],
                        start=(hh == 0), stop=(hh == 1))
                nc.vector.tensor_scalar(qT4[0:KD, i4 * 512:(i4 + 1) * 512],
                                        qkps[0:KD, :], bq_sb[:], None,
                                        op0=ALU.add)
                nc.scalar.activation(kTn[KD:2 * KD, i4 * 512:(i4 + 1) * 512],
                                     qkps[KD:2 * KD, :], AF.Copy)
            # replicate qT to partition groups 1..3 (group 1 also serves the
            # unpacked fallback, whose lhsT kTn lives at partitions 32:64)
            for g in range(1, 4 if PACK_SCORES else 2):
                nc.sync.dma_start(qT4[g * KD:(g + 1) * KD, :], qT4[0:KD, :])
            if PACK_SCORES:
                # regroup kT chunks: kTp[32m:+32, t*128:+128] = kT chunk 4t+m
                kts = kTn[KD:2 * KD, :].rearrange("p (t b) -> p t b", b=4 * P)
                for m in range(4):
                    nc.sync.dma_start(
                        kTp[m * KD:(m + 1) * KD, :].rearrange(
                            "p (t c) -> p t c", c=P),
                        kts[:, :, m * P:(m + 1) * P])

            # --- v (+bv, ones column) ---
            for sc in range(SC):
                vps = sps.tile([P, H], F32, tag="ps")
                for hh in range(HH):
                    nc.tensor.matmul(
                        vps[:],
                        xT[:, hh * S + sc * P: hh * S + (sc + 1) * P],
                        wv_sb[:, hh * H:(hh + 1) * H],
                        start=(hh == 0), stop=(hh == 1))
                nc.vector.tensor_tensor(vall[:, sc * VNP: sc * VNP + H],
                                        vps[:], bvb_sb[:], op=ALU.add)
                nc.gpsimd.memset(vall[:, sc * VNP + H: sc * VNP + VN], 1.0)

        # --- main: scoresT -> exp -> attn@v ---
        # PSUM budget (8 banks): mode 3 -> 3x2-bank score tiles + 2 accs;
        # mode 2 -> 1x4-bank score tile + 4 accs; else 2x2-bank + 4 accs
        sc_bufs = {2: 1, 3: 3}.get(PACK_MODE, 2)
        acc_bufs = 2 if PACK_MODE == 3 else 4
        with tc.tile_pool(name="scps", bufs=sc_bufs, space="PSUM") as scps, \
             tc.tile_pool(name="ops", bufs=acc_bufs, space="PSUM") as ops, \
             tc.tile_pool(name="expool", bufs=int(os.environ.get("EXB", "3"))) as expool, \
             tc.tile_pool(name="outp", bufs=int(os.environ.get("OUB", "2"))) as outp, \
             tc.tile_pool(name="small", bufs=6) as small:
            for ps in range(NPASS):
                accs = [ops.tile([P, VN], F32, tag="acc", name=f"acc{ps}_{ic}")
                        for ic in range(ICP)]
                yall = outp.tile([P, ICP * H], F32, tag="yall")
                for t in range(NQ):  # j-quads
                    # 4 concurrent row-group matmuls; one output per PSUM
                    # bank (modes 2/3); mode 3 uses two 2-bank tiles so the
                    # pool can double-buffer across quads
                    if PACK_MODE == 3:
                        sctiles = [
                            scps.tile([P, 1024], F32, tag="sc",
                                      name=f"scq{ps}_{t}_{h}")
                            for h in range(2)]
                        outs = [sctiles[m // 2][:, (m % 2) * 512:
                                                (m % 2) * 512 + IW]
                                for m in range(4)]
                    elif PACK_MODE == 2:
                        sc_ps = scps.tile([P, 4 * 512], F32, tag="sc")
                        outs = [sc_ps[:, m * 512: m * 512 + IW]
                                for m in range(4)]
                    else:
                        sc_ps = scps.tile([P, 4 * IW], F32, tag="sc")
                        outs = [sc_ps[:, m * IW:(m + 1) * IW]
                                for m in range(4)]
                    for m in range(4):
                        jc = 4 * t + m
                        if PACK_SCORES:
                            nc.tensor.matmul(
                                outs[m],
                                kTp[m * KD:(m + 1) * KD, t * P:(t + 1) * P],
                                qT4[m * KD:(m + 1) * KD,
                                    ps * IW:(ps + 1) * IW],
                                start=True, stop=True,
                                tile_position=(m * KD, 0))
                        else:
                            nc.tensor.matmul(
                                outs[m],
                                kTn[KD:2 * KD, jc * P:(jc + 1) * P],
                                qT4[KD:2 * KD, ps * IW:(ps + 1) * IW],
                                start=True, stop=True)
                    ex = expool.tile([P, 4 * IW], FP8 if DR_MODE else BF16,
                                     tag="ex")
                    ebias = ebias_sb[:] if DR_MODE else 0.0
                    if PACK_MODE == 3:
                        for h in range(2):
                            nc.scalar.activation(
                                ex[:, h * 2 * IW:(h + 1) * 2 * IW].rearrange(
                                    "p (r c) -> p r c", c=IW),
                                sctiles[h].rearrange(
                                    "p (r b) -> p r b", b=512)[:, :, 0:IW],
                                AF.Exp, bias=ebias)
                    elif PACK_MODE == 2:
                        nc.scalar.activation(
                            ex.rearrange("p (m c) -> p m c", c=IW),
                            sc_ps.rearrange("p (m b) -> p m b",
                                            b=512)[:, :, 0:IW],
                            AF.Exp)
                    else:
                        nc.scalar.activation(ex[:], sc_ps[:], AF.Exp)
                    if DR_MODE:
                        for pr in range(2):  # jc pairs within the quad
                            jc0 = 4 * t + 2 * pr
                            lh = ex[:, 2 * pr * IW:(2 * pr + 2) * IW].rearrange(
                                "p (r c) -> p r c", r=2)
                            rh = vall[:, jc0 * VNP:(jc0 + 2) * VNP].rearrange(
                                "p (r c) -> p r c", c=VNP)[:, :, 0:VN]
                            for ic in range(ICP):
                                nc.tensor.matmul(
                                    accs[ic][:],
                                    lh[:, :, ic * P:(ic + 1) * P], rh,
                                    start=(jc0 == 0), stop=(jc0 == SC - 2),
                                    perf_mode=mybir.MatmulPerfMode.DoubleRow)
                    else:
                        for m in range(4):
                            jc = 4 * t + m
                            for ic in range(ICP):
                                nc.tensor.matmul(
                                    accs[ic][:],
                                    ex[:, m * IW + ic * P:
                                       m * IW + (ic + 1) * P],
                                    vall[:, jc * VNP: jc * VNP + VN],
                                    start=(jc == 0), stop=(jc == SC - 1))
                # --- normalize + residual + store (one DMA per pass) ---
                for ic in range(ICP):
                    g = ps * ICP + ic
                    dre = small.tile([P, 1], F32, tag="dre")
                    nc.vector.reciprocal(dre[:], accs[ic][:, H:H + 1])
                    scl = small.tile([P, 1], F32, tag="scl")
                    nc.vector.tensor_tensor(scl[:], dre[:], gmb_sb[:],
                                            op=ALU.mult)
                    yt = outp.tile([P, H], F32, tag="yt")
                    if NORM_MODE == 1:
                        nc.scalar.activation(yt[:], accs[ic][:, 0:H],
                                             AF.Copy, scale=scl[:])
                    else:
                        nc.vector.tensor_scalar(yt[:], accs[ic][:, 0:H],
                                                scl[:], None, op0=ALU.mult)
                    nc.vector.tensor_tensor(yall[:, ic * H:(ic + 1) * H],
                                            yt[:], xall[:, g * H:(g + 1) * H],
                                            op=ALU.add)
                dst = y_d[ps * ICP * P:(ps + 1) * ICP * P, :].rearrange(
                    "(q p) h -> p q h", p=P)
                st_eng = nc.scalar if NORM_MODE == 1 else nc.sync
                st_eng.dma_start(
                    dst, yall.rearrange("p (q h) -> p q h", q=ICP))


def build_program(n_cores: int = 8, reps: int = 1):
    nc = bacc.Bacc("TRN2", target_bir_lowering=False, debug=False,
                   num_devices=n_cores)
    d = {
        "x": nc.dram_tensor("x", [S, H], F32, kind="ExternalInput").ap(),
        "xb": nc.dram_tensor("xb", [S, H], BF16, kind="ExternalInput").ap(),
        "wqk": nc.dram_tensor("wqk", [H, 2 * KD], BF16,
                              kind="ExternalInput").ap(),
        "wv": nc.dram_tensor("wv", [H, H], BF16, kind="ExternalInput").ap(),
        "bq": nc.dram_tensor("bq", [KD, 1], F32, kind="ExternalInput").ap(),
        "bvb": nc.dram_tensor("bvb", [P, H], F32, kind="ExternalInput").ap(),
        "gmb": nc.dram_tensor("gmb", [P, 1], F32, kind="ExternalInput").ap(),
        "idn": nc.dram_tensor("idn", [P, P], F32, kind="ExternalInput").ap(),
        "y": nc.dram_tensor("y", [S, H], F32, kind="ExternalOutput").ap(),
    }
    with tile.TileContext(nc) as tc:
        if reps == 1:
            emit_body(nc, tc, d)
        else:
            with tc.For_i(0, reps, 1):
                emit_body(nc, tc, d)
    nc.compile()
    return nc


_NC = None


def _get_nc():
    global _NC
    if _NC is None:
        _NC = build_program()
    return _NC


def make_in_maps(x, Wq, bq, Wk, bk, Wv, bv, gamma, n_cores=8):
    x = np.asarray(x, np.float32)
    wqk = np.concatenate([np.asarray(Wq, np.float32),
                          np.asarray(Wk, np.float32)], axis=1)
    wqk_b = np.ascontiguousarray(wqk).astype(ml_dtypes.bfloat16)
    wv_b = np.ascontiguousarray(np.asarray(Wv, np.float32)).astype(
        ml_dtypes.bfloat16)
    bq_c = np.ascontiguousarray(np.asarray(bq, np.float32).reshape(KD, 1))
    bvb = np.ascontiguousarray(
        np.broadcast_to(np.asarray(bv, np.float32), (P, H)))
    gmb = np.full((P, 1), np.asarray(gamma, np.float32).reshape(-1)[0],
                  np.float32)
    idn = np.eye(P, dtype=np.float32)
    xb = x.astype(ml_dtypes.bfloat16)
    return [
        {"x": np.ascontiguousarray(x[b]), "xb": np.ascontiguousarray(xb[b]),
         "wqk": wqk_b, "wv": wv_b,
         "bq": bq_c, "bvb": bvb, "gmb": gmb, "idn": idn}
        for b in range(n_cores)
    ]


def kernel(x, Wq, bq, Wk, bk, Wv, bv, gamma):
    nc = _get_nc()
    in_maps = make_in_maps(x, Wq, bq, Wk, bk, Wv, bv, gamma)
    res = run_bass_kernel_spmd(nc, in_maps, list(range(8)))
    return np.stack([res.results[c]["y"] for c in range(8)], axis=0)


# revision 40
# speedup vs baseline: 1.1101x; 1.1101x over previous
"""Trainium2 Bass kernel for nn_Attention_6983616824059.

Single-head attention, B=8, S=2048, H=256, K=32:
    q = x@Wq + bq ; k = x@Wk (+bk cancels in softmax) ; v = x@Wv + bv
    out = gamma * softmax(q k^T) v + x

Sharding: data-parallel over batch, 1 batch element per NeuronCore (8 cores).

Per-core algorithm (PE-facing data bf16, accumulation fp32):
  - load x [2048,256], PE-transpose to xT [256,2048] bf16
  - [qT;kT] = [Wq|Wk]^T xT  (one packed matmul), qT += bq
  - v = x Wv + bv [2048,256] + ones column (gives softmax denom for free)
  - scoresT[j,i] = kT_chunk^T qT : K=32 contraction, 4 j-chunks packed into
    the 128x128 PE array via tile_position row groups (qT replicated to all
    4 partition groups, kT chunks regrouped into kTp)
  - expT = exp(scoresT) (ScalarE, PSUM->SBUF, bf16)
  - out_unnorm = sum_j expT_chunk^T @ v_chunk (PSUM accumulation)
  - y = (gamma / D) * out_unnorm[:, :256] + x
"""

import sys
import numpy as np

sys.path.insert(0, "/opt/trn_rl_repo")

import ml_dtypes  # noqa: E402
import concourse.bass as bass  # noqa: E402
import concourse.tile as tile  # noqa: E402
from concourse import bacc, mybir  # noqa: E402
from concourse.bass_utils import run_bass_kernel_spmd  # noqa: E402

P = 128          # partitions
S = 2048         # sequence
H = 256          # hidden
KD = 32          # q/k head dim
SC = S // P      # 16 s-chunks (j-chunks)
HH = H // P      # 2 h-chunks
IW = 256         # i-slice width per pass
NPASS = S // IW  # 8 passes
ICP = IW // P    # 2 i-chunks per pass
NQ = SC // 4     # 4 j-quads per pass
VN = H + 2       # v free width: 256 + ones col + pad (col 257 = dup ones)

F32 = mybir.dt.float32
BF16 = mybir.dt.bfloat16
AF = mybir.ActivationFunctionType
ALU = mybir.AluOpType

import os
# 0 = no packing, 1 = packed + two outputs per PSUM bank,
# 2 = packed + one output per PSUM bank (strided exp read),
# 3 = packed + one output per bank, two 2-bank tiles per quad (pipelined)
PACK_MODE = int(os.environ.get("PACK_MODE", "3"))
PACK_SCORES = PACK_MODE > 0
# 0 = PE transpose of f32 x, 1 = DMA xbar transpose of host-cast bf16 x
TR_MODE = int(os.environ.get("TR_MODE", "1"))
# 1 = scale-step of the normalization on ScalarE + y stores on the ACT
# HWDGE ring (splits the pass-end latency chain across engines)
NORM_MODE = int(os.environ.get("NORM_MODE", "2"))
# 1 = fp8e5 DoubleRow attn@v: exp output and v in fp8e5, two j-chunks
# contracted per matmul (128x256 virtual array). exp biased by -2 for
# range margin (softmax-invariant). Needs VN padded so the chunk stride
# is a multiple of 16 bytes.
DR_MODE = int(os.environ.get("DR_MODE", "0"))
VNP = 272 if DR_MODE else VN
FP8 = mybir.dt.float8e5
# 1 = software-pipelined emission: scores of quad g+1 precede attn of
# quad g in the PE queue, hiding the exp(g) wait
PIPE = int(os.environ.get("PIPE", "1"))


def emit_body(nc, tc, d):
    x_d, wqk_d, wv_d, bq_d, bvb_d, gmb_d, idn_d, y_d = (
        d["x"], d["wqk"], d["wv"], d["bq"], d["bvb"], d["gmb"], d["idn"],
        d["y"])
    xb_d = d["xb"]

    with tc.tile_pool(name="const", bufs=1) as const, \
         tc.tile_pool(name="big", bufs=1) as big:
        # --- constants ---
        wqk_sb = const.tile([P, HH * 2 * KD], BF16)  # h-chunk hh at [:, hh*64:]
        wv_sb = const.tile([P, HH * H], BF16)        # h-chunk hh at [:, hh*H:]
        bq_sb = const.tile([KD, 1], F32)
        bvb_sb = const.tile([P, H], F32)
        gmb_sb = const.tile([P, 1], F32)
        idn_sb = const.tile([P, P], F32)
        for hh in range(HH):
            nc.sync.dma_start(wqk_sb[:, hh * 2 * KD:(hh + 1) * 2 * KD],
                              wqk_d[hh * P:(hh + 1) * P, :])
            nc.sync.dma_start(wv_sb[:, hh * H:(hh + 1) * H],
                              wv_d[hh * P:(hh + 1) * P, :])
        ebias_sb = const.tile([P, 1], F32)
        nc.gpsimd.memset(ebias_sb[:], -2.0)
        nc.sync.dma_start(bq_sb[:], bq_d[:])
        nc.sync.dma_start(bvb_sb[:], bvb_d[:])
        nc.sync.dma_start(gmb_sb[:], gmb_d[:])
        nc.sync.dma_start(idn_sb[:], idn_d[:])

        # --- resident tensors ---
        xall = big.tile([P, SC * H], F32)      # s-chunk sc at [:, sc*H:]
        xT = big.tile([P, HH * S], BF16)       # h-chunk hh at [:, hh*S + s]
        qT4 = big.tile([P, S], BF16)           # qT replicated to 4 part-groups
        kTn = big.tile([2 * KD, S], BF16)      # kT in rows 32:64
        kTp = big.tile([P, NQ * P], BF16)      # kTp[32m+p, t*128+c] = chunk 4t+m
        vall = big.tile([P, SC * VNP], FP8 if DR_MODE else BF16)

        # --- load x (4 batched DMAs) ---
        for g in range(4):
            src = x_d[g * 4 * P:(g + 1) * 4 * P, :].rearrange(
                "(q p) h -> p q h", p=P)
            dst = xall[:, g * 4 * H:(g + 1) * 4 * H].rearrange(
                "p (q h) -> p q h", q=4)
            nc.sync.dma_start(dst, src)

        with tc.tile_pool(name="sps", bufs=4, space="PSUM") as sps:
            xT3 = xT.rearrange("p (hh s) -> p hh s", hh=HH)
            if TR_MODE == 1:
                # xbar DMA transpose straight from DRAM (bf16 copy of x)
                xb3 = xb_d.rearrange("s (hh c) -> s hh c", c=P)
                for hh in range(HH):
                    nc.sync.dma_start(out=xT3[:, hh], in_=xb3[:, hh],
                                      transpose=True)
            else:
                # PE transpose of f32 x; PSUM->SBUF cast copies on DVE/ACT
                for sc in range(SC):
                    tr = sps.tile([P, H], F32, tag="ps")
                    for hh in range(HH):
                        nc.tensor.transpose(
                            tr[:, hh * P:(hh + 1) * P],
                            xall[:, sc * H + hh * P: sc * H + (hh + 1) * P],
                            idn_sb[:])
                    eng = nc.vector.tensor_copy if sc % 2 == 0 else (
                        lambda o, i: nc.scalar.activation(o, i, AF.Copy))
                    eng(xT3[:, :, sc * P:(sc + 1) * P],
                        tr.rearrange("p (hh c) -> p hh c", hh=HH))

            # --- qT / kT (one packed matmul per 512-slice) ---
            for i4 in range(4):
                qkps = sps.tile([2 * KD, 512], F32, tag="ps")
                for hh in range(HH):
                    nc.tensor.matmul(
                        qkps[:], wqk_sb[:, hh * 2 * KD:(hh + 1) * 2 * KD],
                        xT[:, hh * S + i4 * 512: hh * S + (i4 + 1) * 512],
                        start=(hh == 0), stop=(hh == 1))
                nc.vector.tensor_scalar(qT4[0:KD, i4 * 512:(i4 + 1) * 512],
                                        qkps[0:KD, :], bq_sb[:], None,
                                        op0=ALU.add)
                nc.scalar.activation(kTn[KD:2 * KD, i4 * 512:(i4 + 1) * 512],
                                     qkps[KD:2 * KD, :], AF.Copy)
            # replicate qT to partition groups 1..3 (group 1 also serves the
            # unpacked fallback, whose lhsT kTn lives at partitions 32:64)
            for g in range(1, 4 if PACK_SCORES else 2):
                nc.sync.dma_start(qT4[g * KD:(g + 1) * KD, :], qT4[0:KD, :])
            if PACK_SCORES:
                # regroup kT chunks: kTp[32m:+32, t*128:+128] = kT chunk 4t+m
                kts = kTn[KD:2 * KD, :].rearrange("p (t b) -> p t b", b=4 * P)
                for m in range(4):
                    nc.sync.dma_start(
                        kTp[m * KD:(m + 1) * KD, :].rearrange(
                            "p (t c) -> p t c", c=P),
                        kts[:, :, m * P:(m + 1) * P])

            # --- v (+bv, ones column) ---
            for sc in range(SC):
                vps = sps.tile([P, H], F32, tag="ps")
                for hh in range(HH):
                    nc.tensor.matmul(
                        vps[:],
                        xT[:, hh * S + sc * P: hh * S + (sc + 1) * P],
                        wv_sb[:, hh * H:(hh + 1) * H],
                        start=(hh == 0), stop=(hh == 1))
                nc.vector.tensor_tensor(vall[:, sc * VNP: sc * VNP + H],
                                        vps[:], bvb_sb[:], op=ALU.add)
                nc.gpsimd.memset(vall[:, sc * VNP + H: sc * VNP + VN], 1.0)

        # --- main: scoresT -> exp -> attn@v ---
        # PSUM budget (8 banks): mode 3 -> 3x2-bank score tiles + 2 accs;
        # mode 2 -> 1x4-bank score tile + 4 accs; else 2x2-bank + 4 accs
        sc_bufs = {2: 1, 3: 3}.get(PACK_MODE, 2)
        acc_bufs = 2 if PACK_MODE == 3 else 4
        with tc.tile_pool(name="scps", bufs=sc_bufs, space="PSUM") as scps, \
             tc.tile_pool(name="ops", bufs=acc_bufs, space="PSUM") as ops, \
             tc.tile_pool(name="expool", bufs=int(os.environ.get("EXB", "3"))) as expool, \
             tc.tile_pool(name="outp", bufs=int(os.environ.get("OUB", "2"))) as outp, \
             tc.tile_pool(name="small", bufs=6) as small:
            # Software-pipelined emission (PIPE=1): scores for quad g+1 are
            # emitted BEFORE the attn matmuls of quad g, so the in-order PE
            # queue can compute them while ACT runs exp(g) instead of
            # head-of-line blocking on it.
            NGQ = NPASS * NQ
            accs_by_ps = {}
            yall_by_ps = {}
            sct = {}
            exs = {}

            def emit_scores(gq):
                ps, t = divmod(gq, NQ)
                if t == 0:
                    accs_by_ps[ps] = [
                        ops.tile([P, VN], F32, tag="acc",
                                 name=f"acc{ps}_{ic}") for ic in range(ICP)]
                    yall_by_ps[ps] = outp.tile([P, ICP * H], F32, tag="yall",
                                               name=f"yall{ps}")
                if PACK_MODE == 3:
                    tiles = [scps.tile([P, 1024], F32, tag="sc",
                                       name=f"scq{ps}_{t}_{h}")
                             for h in range(2)]
                    outs = [tiles[m // 2][:, (m % 2) * 512:(m % 2) * 512 + IW]
                            for m in range(4)]
                elif PACK_MODE == 2:
                    tiles = [scps.tile([P, 4 * 512], F32, tag="sc",
                                       name=f"scq{ps}_{t}")]
                    outs = [tiles[0][:, m * 512: m * 512 + IW]
                            for m in range(4)]
                else:
                    tiles = [scps.tile([P, 4 * IW], F32, tag="sc",
                                       name=f"scq{ps}_{t}")]
                    outs = [tiles[0][:, m * IW:(m + 1) * IW] for m in range(4)]
                sct[gq] = tiles
                for m in range(4):
                    jc = 4 * t + m
                    if PACK_SCORES:
                        nc.tensor.matmul(
                            outs[m],
                            kTp[m * KD:(m + 1) * KD, t * P:(t + 1) * P],
                            qT4[m * KD:(m + 1) * KD, ps * IW:(ps + 1) * IW],
                            start=True, stop=True, tile_position=(m * KD, 0))
                    else:
                        nc.tensor.matmul(
                            outs[m],
                            kTn[KD:2 * KD, jc * P:(jc + 1) * P],
                            qT4[KD:2 * KD, ps * IW:(ps + 1) * IW],
                            start=True, stop=True)

            def emit_exp(gq):
                tiles = sct.pop(gq)
                ex = expool.tile([P, 4 * IW], FP8 if DR_MODE else BF16,
                                 tag="ex", name=f"ex{gq}")
                exs[gq] = ex
                ebias = ebias_sb[:] if DR_MODE else 0.0
                if PACK_MODE == 3:
                    for h in range(2):
                        nc.scalar.activation(
                            ex[:, h * 2 * IW:(h + 1) * 2 * IW].rearrange(
                                "p (r c) -> p r c", c=IW),
                            tiles[h].rearrange(
                                "p (r b) -> p r b", b=512)[:, :, 0:IW],
                            AF.Exp, bias=ebias)
                elif PACK_MODE == 2:
                    nc.scalar.activation(
                        ex.rearrange("p (m c) -> p m c", c=IW),
                        tiles[0].rearrange("p (m b) -> p m b",
                                           b=512)[:, :, 0:IW], AF.Exp)
                else:
                    nc.scalar.activation(ex[:], tiles[0][:], AF.Exp)

            def emit_attn(gq):
                ps, t = divmod(gq, NQ)
                accs = accs_by_ps[ps]
                ex = exs.pop(gq)
                if DR_MODE:
                    for pr in range(2):  # jc pairs within the quad
                        jc0 = 4 * t + 2 * pr
                        lh = ex[:, 2 * pr * IW:(2 * pr + 2) * IW].rearrange(
                            "p (r c) -> p r c", r=2)
                        rh = vall[:, jc0 * VNP:(jc0 + 2) * VNP].rearrange(
                            "p (r c) -> p r c", c=VNP)[:, :, 0:VN]
                        for ic in range(ICP):
                            nc.tensor.matmul(
                                accs[ic][:], lh[:, :, ic * P:(ic + 1) * P],
                                rh, start=(jc0 == 0), stop=(jc0 == SC - 2),
                                perf_mode=mybir.MatmulPerfMode.DoubleRow)
                else:
                    for m in range(4):
                        jc = 4 * t + m
                        for ic in range(ICP):
                            nc.tensor.matmul(
                                accs[ic][:],
                                ex[:, m * IW + ic * P: m * IW + (ic + 1) * P],
                                vall[:, jc * VNP: jc * VNP + VN],
                                start=(jc == 0), stop=(jc == SC - 1))

            def emit_norm(ps):
                # normalize + residual + store (one DMA per pass).
                # PSUM-freeing ops (reciprocal + scale-mult from accs) go
                # first so the acc slots release for the next pass ASAP; the
                # SBUF-only residual adds run on the otherwise-idle Pool
                # engine (NORM_MODE 2) or DVE.
                accs = accs_by_ps.pop(ps)
                yall = yall_by_ps.pop(ps)
                yts = []
                for ic in range(ICP):
                    dre = small.tile([P, 1], F32, tag="dre",
                                     name=f"dre{ps}_{ic}")
                    nc.vector.reciprocal(dre[:], accs[ic][:, H:H + 1])
                    scl = small.tile([P, 1], F32, tag="scl",
                                     name=f"scl{ps}_{ic}")
                    nc.vector.tensor_tensor(scl[:], dre[:], gmb_sb[:],
                                            op=ALU.mult)
                    yt = outp.tile([P, H], F32, tag="yt", name=f"yt{ps}_{ic}")
                    if NORM_MODE == 1:
                        nc.scalar.activation(yt[:], accs[ic][:, 0:H],
                                             AF.Copy, scale=scl[:])
                    else:
                        nc.vector.tensor_scalar(yt[:], accs[ic][:, 0:H],
                                                scl[:], None, op0=ALU.mult)
                    yts.append(yt)
                add_eng = nc.gpsimd if NORM_MODE == 2 else nc.vector
                for ic in range(ICP):
                    g = ps * ICP + ic
                    add_eng.tensor_tensor(yall[:, ic * H:(ic + 1) * H],
                                          yts[ic][:],
                                          xall[:, g * H:(g + 1) * H],
                                          op=ALU.add)
                dst = y_d[ps * ICP * P:(ps + 1) * ICP * P, :].rearrange(
                    "(q p) h -> p q h", p=P)
                st_eng = nc.scalar if NORM_MODE == 1 else nc.sync
                st_eng.dma_start(
                    dst, yall.rearrange("p (q h) -> p q h", q=ICP))

            if PIPE:
                emit_scores(0)
                for gq in range(NGQ):
                    emit_exp(gq)
                    if gq + 1 < NGQ:
                        emit_scores(gq + 1)
                    emit_attn(gq)
                    if (gq + 1) % NQ == 0:
                        emit_norm(gq // NQ)
            else:
                for gq in range(NGQ):
                    emit_scores(gq)
                    emit_exp(gq)
                    emit_attn(gq)
                    if (gq + 1) % NQ == 0:
                        emit_norm(gq // NQ)


def build_program(n_cores: int = 8, reps: int = 1):
    nc = bacc.Bacc("TRN2", target_bir_lowering=False, debug=False,
                   num_devices=n_cores)
    d = {
        "x": nc.dram_tensor("x", [S, H], F32, kind="ExternalInput").ap(),
        "xb": nc.dram_tensor("xb", [S, H], BF16, kind="ExternalInput").ap(),
        "wqk": nc.dram_tensor("wqk", [H, 2 * KD], BF16,
                              kind="ExternalInput").ap(),
        "wv": nc.dram_tensor("wv", [H, H], BF16, kind="ExternalInput").ap(),
        "bq": nc.dram_tensor("bq", [KD, 1], F32, kind="ExternalInput").ap(),
        "bvb": nc.dram_tensor("bvb", [P, H], F32, kind="ExternalInput").ap(),
        "gmb": nc.dram_tensor("gmb", [P, 1], F32, kind="ExternalInput").ap(),
        "idn": nc.dram_tensor("idn", [P, P], F32, kind="ExternalInput").ap(),
        "y": nc.dram_tensor("y", [S, H], F32, kind="ExternalOutput").ap(),
    }
    with tile.TileContext(nc) as tc:
        if reps == 1:
            emit_body(nc, tc, d)
        else:
            with tc.For_i(0, reps, 1):
                emit_body(nc, tc, d)
    nc.compile()
    return nc


_NC = None


def _get_nc():
    global _NC
    if _NC is None:
        _NC = build_program()
    return _NC


def make_in_maps(x, Wq, bq, Wk, bk, Wv, bv, gamma, n_cores=8):
    x = np.asarray(x, np.float32)
    wqk = np.concatenate([np.asarray(Wq, np.float32),
                          np.asarray(Wk, np.float32)], axis=1)
    wqk_b = np.ascontiguousarray(wqk).astype(ml_dtypes.bfloat16)
    wv_b = np.ascontiguousarray(np.asarray(Wv, np.float32)).astype(
        ml_dtypes.bfloat16)
    bq_c = np.ascontiguousarray(np.asarray(bq, np.float32).reshape(KD, 1))
    bvb = np.ascontiguousarray(
        np.broadcast_to(np.asarray(bv, np.float32), (P, H)))
    gmb = np.full((P, 1), np.asarray(gamma, np.float32).reshape(-1)[0],
                  np.float32)
    idn = np.eye(P, dtype=np.float32)
    xb = x.astype(ml_dtypes.bfloat16)
    return [
        {"x": np.ascontiguousarray(x[b]), "xb": np.ascontiguousarray(xb[b]),
         "wqk": wqk_b, "wv": wv_b,
         "bq": bq_c, "bvb": bvb, "gmb": gmb, "idn": idn}
        for b in range(n_cores)
    ]


def kernel(x, Wq, bq, Wk, bk, Wv, bv, gamma):
    nc = _get_nc()
    in_maps = make_in_maps(x, Wq, bq, Wk, bk, Wv, bv, gamma)
    res = run_bass_kernel_spmd(nc, in_maps, list(range(8)))
    return np.stack([res.results[c]["y"] for c in range(8)], axis=0)


# revision 47
# speedup vs baseline: 1.2189x; 1.0980x over previous
"""Trainium2 Bass kernel for nn_Attention_6983616824059.

Single-head attention, B=8, S=2048, H=256, K=32:
    q = x@Wq + bq ; k = x@Wk (+bk cancels in softmax) ; v = x@Wv + bv
    out = gamma * softmax(q k^T) v + x

Sharding: data-parallel over batch, 1 batch element per NeuronCore (8 cores).

Per-core algorithm (PE-facing data bf16, accumulation fp32):
  - xT [256,2048] bf16 via DMA xbar transpose straight from DRAM (from a
    host-cast bf16 copy of x); f32 x loads in parallel for the residual
  - [qT;kT] = [Wq|Wk]^T xT  (one packed matmul), qT += bq; bk dropped
    (provably cancels in softmax)
  - v = x Wv + bv [2048,256] + ones column (gives softmax denom for free)
  - scoresT[j,i] = kT_chunk^T qT : K=32 contraction, 4 j-chunks packed
    into the 128x128 PE array via tile_position row groups (qT replicated
    to all 4 partition groups, kT chunks regrouped into kTp); each packed
    matmul MUST land in its own PSUM bank (same-bank concurrency faults
    the device); two 2-bank score tiles per quad, pool bufs=3, so scores
    of quad g+1 overlap exp(g)
  - expT = exp(scoresT) (ScalarE, PSUM->SBUF, bf16); ScalarE does exp
    ONLY - loading it with anything else measured +16 us
  - out_unnorm = sum_j expT_chunk^T @ v_chunk (PSUM accumulation, 2
    accumulators per 256-wide i-pass)
  - y = (gamma / D) * out_unnorm[:, :256] + x : reciprocal+scale on DVE
    (frees the acc PSUM banks ASAP), residual adds on the idle Pool
    engine (-14 us), one batched y DMA per pass
"""

import sys
import numpy as np

sys.path.insert(0, "/opt/trn_rl_repo")

import ml_dtypes  # noqa: E402
import concourse.bass as bass  # noqa: E402
import concourse.tile as tile  # noqa: E402
from concourse import bacc, mybir  # noqa: E402
from concourse.bass_utils import run_bass_kernel_spmd  # noqa: E402

P = 128          # partitions
S = 2048         # sequence
H = 256          # hidden
KD = 32          # q/k head dim
SC = S // P      # 16 s-chunks (j-chunks)
HH = H // P      # 2 h-chunks
import os
IW = int(os.environ.get("IW", "256"))  # i-slice width per pass
NPASS = S // IW  # passes
ICP = IW // P    # i-chunks per pass
NQ = SC // 4     # 4 j-quads per pass
VN = H + 2       # v free width: 256 + ones col + pad (col 257 = dup ones)

F32 = mybir.dt.float32
BF16 = mybir.dt.bfloat16
AF = mybir.ActivationFunctionType
ALU = mybir.AluOpType

# 0 = no packing, 1 = packed + two outputs per PSUM bank,
# 2 = packed + one output per PSUM bank (strided exp read),
# 3 = packed + one output per bank, two 2-bank tiles per quad (pipelined)
PACK_MODE = int(os.environ.get("PACK_MODE", "3"))
PACK_SCORES = PACK_MODE > 0
# 0 = PE transpose of f32 x, 1 = DMA xbar transpose of host-cast bf16 x
TR_MODE = int(os.environ.get("TR_MODE", "1"))
# 1 = scale-step of the normalization on ScalarE + y stores on the ACT
# HWDGE ring (splits the pass-end latency chain across engines)
NORM_MODE = int(os.environ.get("NORM_MODE", "2"))
# 1 = fp8e5 DoubleRow attn@v: exp output and v in fp8e5, two j-chunks
# contracted per matmul (128x256 virtual array). exp biased by -2 for
# range margin (softmax-invariant). Needs VN padded so the chunk stride
# is a multiple of 16 bytes.
DR_MODE = int(os.environ.get("DR_MODE", "0"))
VNP = 272 if DR_MODE else VN
FP8 = mybir.dt.float8e5
# 1 = software-pipelined emission: scores of quad g+1 precede attn of
# quad g in the PE queue, hiding the exp(g) wait
PIPE = int(os.environ.get("PIPE", "1"))


def emit_body(nc, tc, d):
    x_d, wqk_d, wv_d, bq_d, bvb_d, gmb_d, idn_d, y_d = (
        d["x"], d["wqk"], d["wv"], d["bq"], d["bvb"], d["gmb"], d["idn"],
        d["y"])
    xb_d = d["xb"]

    with tc.tile_pool(name="const", bufs=1) as const, \
         tc.tile_pool(name="big", bufs=1) as big:
        # --- constants ---
        wqk_sb = const.tile([P, HH * 2 * KD], BF16)  # h-chunk hh at [:, hh*64:]
        wv_sb = const.tile([P, HH * H], BF16)        # h-chunk hh at [:, hh*H:]
        bq_sb = const.tile([KD, 1], F32)
        bvb_sb = const.tile([P, H], F32)
        gmb_sb = const.tile([P, 1], F32)
        idn_sb = const.tile([P, P], F32)
        # --- resident tensors ---
        xall = big.tile([P, SC * H], F32)      # s-chunk sc at [:, sc*H:]
        xT = big.tile([P, HH * S], BF16)       # h-chunk hh at [:, hh*S + s]
        qT4 = big.tile([P, S], BF16)           # qT replicated to 4 part-groups
        kTn = big.tile([2 * KD, S], BF16)      # kT in rows 32:64
        kTp = big.tile([P, NQ * P], BF16)      # kTp[32m+p, t*128+c] = chunk 4t+m
        vall = big.tile([P, SC * VNP], FP8 if DR_MODE else BF16)

        # DMA emission order = HWDGE FIFO order: put the xbar transposes
        # and the qk weights (the critical path to the first scores matmul)
        # ahead of the bulk f32 x load and late-needed constants.
        xT3 = xT.rearrange("p (hh s) -> p hh s", hh=HH)
        if TR_MODE == 1:
            # xbar DMA transpose straight from DRAM (bf16 copy of x)
            xb3 = xb_d.rearrange("s (hh c) -> s hh c", c=P)
            for hh in range(HH):
                nc.sync.dma_start(out=xT3[:, hh], in_=xb3[:, hh],
                                  transpose=True)
        for hh in range(HH):
            nc.sync.dma_start(wqk_sb[:, hh * 2 * KD:(hh + 1) * 2 * KD],
                              wqk_d[hh * P:(hh + 1) * P, :])
        nc.sync.dma_start(bq_sb[:], bq_d[:])
        for hh in range(HH):
            nc.sync.dma_start(wv_sb[:, hh * H:(hh + 1) * H],
                              wv_d[hh * P:(hh + 1) * P, :])
        ebias_sb = const.tile([P, 1], F32)
        nc.gpsimd.memset(ebias_sb[:], -2.0)
        nc.sync.dma_start(bvb_sb[:], bvb_d[:])
        nc.sync.dma_start(gmb_sb[:], gmb_d[:])
        nc.sync.dma_start(idn_sb[:], idn_d[:])

        # --- load x (4 batched DMAs; only needed by the residual adds) ---
        for g in range(4):
            src = x_d[g * 4 * P:(g + 1) * 4 * P, :].rearrange(
                "(q p) h -> p q h", p=P)
            dst = xall[:, g * 4 * H:(g + 1) * 4 * H].rearrange(
                "p (q h) -> p q h", q=4)
            nc.sync.dma_start(dst, src)

        with tc.tile_pool(name="sps", bufs=4, space="PSUM") as sps:
            if TR_MODE == 1:
                pass  # xT already produced above by the xbar transposes
            else:
                # PE transpose of f32 x; PSUM->SBUF cast copies on DVE/ACT
                for sc in range(SC):
                    tr = sps.tile([P, H], F32, tag="ps")
                    for hh in range(HH):
                        nc.tensor.transpose(
                            tr[:, hh * P:(hh + 1) * P],
                            xall[:, sc * H + hh * P: sc * H + (hh + 1) * P],
                            idn_sb[:])
                    eng = nc.vector.tensor_copy if sc % 2 == 0 else (
                        lambda o, i: nc.scalar.activation(o, i, AF.Copy))
                    eng(xT3[:, :, sc * P:(sc + 1) * P],
                        tr.rearrange("p (hh c) -> p hh c", hh=HH))

            # --- qT / kT (one packed matmul per 512-slice) ---
            for i4 in range(4):
                qkps = sps.tile([2 * KD, 512], F32, tag="ps")
                for hh in range(HH):
                    nc.tensor.matmul(
                        qkps[:], wqk_sb[:, hh * 2 * KD:(hh + 1) * 2 * KD],
                        xT[:, hh * S + i4 * 512: hh * S + (i4 + 1) * 512],
                        start=(hh == 0), stop=(hh == 1))
                nc.vector.tensor_scalar(qT4[0:KD, i4 * 512:(i4 + 1) * 512],
                                        qkps[0:KD, :], bq_sb[:], None,
                                        op0=ALU.add)
                nc.scalar.activation(kTn[KD:2 * KD, i4 * 512:(i4 + 1) * 512],
                                     qkps[KD:2 * KD, :], AF.Copy)
            # replicate qT to partition groups 1..3 (group 1 also serves the
            # unpacked fallback, whose lhsT kTn lives at partitions 32:64)
            for g in range(1, 4 if PACK_SCORES else 2):
                nc.sync.dma_start(qT4[g * KD:(g + 1) * KD, :], qT4[0:KD, :])
            if PACK_SCORES:
                # regroup kT chunks: kTp[32m:+32, t*128:+128] = kT chunk 4t+m
                kts = kTn[KD:2 * KD, :].rearrange("p (t b) -> p t b", b=4 * P)
                for m in range(4):
                    nc.sync.dma_start(
                        kTp[m * KD:(m + 1) * KD, :].rearrange(
                            "p (t c) -> p t c", c=P),
                        kts[:, :, m * P:(m + 1) * P])

            # --- v (+bv, ones column) ---
            for sc in range(SC):
                vps = sps.tile([P, H], F32, tag="ps")
                for hh in range(HH):
                    nc.tensor.matmul(
                        vps[:],
                        xT[:, hh * S + sc * P: hh * S + (sc + 1) * P],
                        wv_sb[:, hh * H:(hh + 1) * H],
                        start=(hh == 0), stop=(hh == 1))
                nc.vector.tensor_tensor(vall[:, sc * VNP: sc * VNP + H],
                                        vps[:], bvb_sb[:], op=ALU.add)
                nc.gpsimd.memset(vall[:, sc * VNP + H: sc * VNP + VN], 1.0)

        # --- main: scoresT -> exp -> attn@v ---
        # PSUM budget (8 banks): mode 3 -> 3x2-bank score tiles + 2 accs
        # (IW=256), or 2x2-bank tiles + 4 accs (IW=512);
        # mode 2 -> 1x4-bank score tile + 4 accs; else 2x2-bank + 4 accs
        sc_bufs = {2: 1, 3: 3}.get(PACK_MODE, 2)
        acc_bufs = 2 if PACK_MODE == 3 else 4
        if ICP == 4:
            sc_bufs, acc_bufs = 2, 4
        with tc.tile_pool(name="scps", bufs=sc_bufs, space="PSUM") as scps, \
             tc.tile_pool(name="ops", bufs=acc_bufs, space="PSUM") as ops, \
             tc.tile_pool(name="expool", bufs=int(os.environ.get("EXB", "3"))) as expool, \
             tc.tile_pool(name="outp", bufs=int(os.environ.get("OUB", "2"))) as outp, \
             tc.tile_pool(name="small", bufs=6) as small:
            # Software-pipelined emission (PIPE=1): scores for quad g+1 are
            # emitted BEFORE the attn matmuls of quad g, so the in-order PE
            # queue can compute them while ACT runs exp(g) instead of
            # head-of-line blocking on it.
            NGQ = NPASS * NQ
            accs_by_ps = {}
            yall_by_ps = {}
            sct = {}
            exs = {}

            def emit_scores(gq):
                ps, t = divmod(gq, NQ)
                if t == 0:
                    accs_by_ps[ps] = [
                        ops.tile([P, VN], F32, tag="acc",
                                 name=f"acc{ps}_{ic}") for ic in range(ICP)]
                    yall_by_ps[ps] = outp.tile([P, ICP * H], F32, tag="yall",
                                               name=f"yall{ps}")
                if PACK_MODE == 3:
                    tiles = [scps.tile([P, 1024], F32, tag="sc",
                                       name=f"scq{ps}_{t}_{h}")
                             for h in range(2)]
                    outs = [tiles[m // 2][:, (m % 2) * 512:(m % 2) * 512 + IW]
                            for m in range(4)]
                elif PACK_MODE == 2:
                    tiles = [scps.tile([P, 4 * 512], F32, tag="sc",
                                       name=f"scq{ps}_{t}")]
                    outs = [tiles[0][:, m * 512: m * 512 + IW]
                            for m in range(4)]
                else:
                    tiles = [scps.tile([P, 4 * IW], F32, tag="sc",
                                       name=f"scq{ps}_{t}")]
                    outs = [tiles[0][:, m * IW:(m + 1) * IW] for m in range(4)]
                sct[gq] = tiles
                for m in range(4):
                    jc = 4 * t + m
                    if PACK_SCORES:
                        nc.tensor.matmul(
                            outs[m],
                            kTp[m * KD:(m + 1) * KD, t * P:(t + 1) * P],
                            qT4[m * KD:(m + 1) * KD, ps * IW:(ps + 1) * IW],
                            start=True, stop=True, tile_position=(m * KD, 0))
                    else:
                        nc.tensor.matmul(
                            outs[m],
                            kTn[KD:2 * KD, jc * P:(jc + 1) * P],
                            qT4[KD:2 * KD, ps * IW:(ps + 1) * IW],
                            start=True, stop=True)

            def emit_exp(gq):
                tiles = sct.pop(gq)
                ex = expool.tile([P, 4 * IW], FP8 if DR_MODE else BF16,
                                 tag="ex", name=f"ex{gq}")
                exs[gq] = ex
                ebias = ebias_sb[:] if DR_MODE else 0.0
                if PACK_MODE == 3:
                    for h in range(2):
                        nc.scalar.activation(
                            ex[:, h * 2 * IW:(h + 1) * 2 * IW].rearrange(
                                "p (r c) -> p r c", c=IW),
                            tiles[h].rearrange(
                                "p (r b) -> p r b", b=512)[:, :, 0:IW],
                            AF.Exp, bias=ebias)
                elif PACK_MODE == 2:
                    nc.scalar.activation(
                        ex.rearrange("p (m c) -> p m c", c=IW),
                        tiles[0].rearrange("p (m b) -> p m b",
                                           b=512)[:, :, 0:IW], AF.Exp)
                else:
                    nc.scalar.activation(ex[:], tiles[0][:], AF.Exp)

            def emit_attn(gq):
                ps, t = divmod(gq, NQ)
                accs = accs_by_ps[ps]
                ex = exs.pop(gq)
                if DR_MODE:
                    for pr in range(2):  # jc pairs within the quad
                        jc0 = 4 * t + 2 * pr
                        lh = ex[:, 2 * pr * IW:(2 * pr + 2) * IW].rearrange(
                            "p (r c) -> p r c", r=2)
                        rh = vall[:, jc0 * VNP:(jc0 + 2) * VNP].rearrange(
                            "p (r c) -> p r c", c=VNP)[:, :, 0:VN]
                        for ic in range(ICP):
                            nc.tensor.matmul(
                                accs[ic][:], lh[:, :, ic * P:(ic + 1) * P],
                                rh, start=(jc0 == 0), stop=(jc0 == SC - 2),
                                perf_mode=mybir.MatmulPerfMode.DoubleRow)
                else:
                    for m in range(4):
                        jc = 4 * t + m
                        for ic in range(ICP):
                            nc.tensor.matmul(
                                accs[ic][:],
                                ex[:, m * IW + ic * P: m * IW + (ic + 1) * P],
                                vall[:, jc * VNP: jc * VNP + VN],
                                start=(jc == 0), stop=(jc == SC - 1))

            def emit_norm(ps):
                # normalize + residual + store (one DMA per pass).
                # PSUM-freeing ops (reciprocal + scale-mult from accs) go
                # first so the acc slots release for the next pass ASAP; the
                # SBUF-only residual adds run on the otherwise-idle Pool
                # engine (NORM_MODE 2) or DVE.
                accs = accs_by_ps.pop(ps)
                yall = yall_by_ps.pop(ps)
                # gamma is folded into Wv/bv on the host, so the scale is
                # just 1/D
                yts = []
                for ic in range(ICP):
                    dre = small.tile([P, 1], F32, tag="dre",
                                     name=f"dre{ps}_{ic}")
                    nc.vector.reciprocal(dre[:], accs[ic][:, H:H + 1])
                    yt = outp.tile([P, H], F32, tag="yt", name=f"yt{ps}_{ic}")
                    if NORM_MODE == 1:
                        nc.scalar.activation(yt[:], accs[ic][:, 0:H],
                                             AF.Copy, scale=dre[:])
                    else:
                        nc.vector.tensor_scalar(yt[:], accs[ic][:, 0:H],
                                                dre[:], None, op0=ALU.mult)
                    yts.append(yt)
                add_eng = nc.gpsimd if NORM_MODE == 2 else nc.vector
                for ic in range(ICP):
                    g = ps * ICP + ic
                    add_eng.tensor_tensor(yall[:, ic * H:(ic + 1) * H],
                                          yts[ic][:],
                                          xall[:, g * H:(g + 1) * H],
                                          op=ALU.add)
                dst = y_d[ps * ICP * P:(ps + 1) * ICP * P, :].rearrange(
                    "(q p) h -> p q h", p=P)
                st_eng = nc.scalar if NORM_MODE == 1 else nc.sync
                st_eng.dma_start(
                    dst, yall.rearrange("p (q h) -> p q h", q=ICP))

            if PIPE:
                emit_scores(0)
                for gq in range(NGQ):
                    emit_exp(gq)
                    if gq + 1 < NGQ:
                        emit_scores(gq + 1)
                    emit_attn(gq)
                    if (gq + 1) % NQ == 0:
                        emit_norm(gq // NQ)
            else:
                for gq in range(NGQ):
                    emit_scores(gq)
                    emit_exp(gq)
                    emit_attn(gq)
                    if (gq + 1) % NQ == 0:
                        emit_norm(gq // NQ)


def build_program(n_cores: int = 8, reps: int = 1):
    nc = bacc.Bacc("TRN2", target_bir_lowering=False, debug=False,
                   num_devices=n_cores)
    d = {
        "x": nc.dram_tensor("x", [S, H], F32, kind="ExternalInput").ap(),
        "xb": nc.dram_tensor("xb", [S, H], BF16, kind="ExternalInput").ap(),
        "wqk": nc.dram_tensor("wqk", [H, 2 * KD], BF16,
                              kind="ExternalInput").ap(),
        "wv": nc.dram_tensor("wv", [H, H], BF16, kind="ExternalInput").ap(),
        "bq": nc.dram_tensor("bq", [KD, 1], F32, kind="ExternalInput").ap(),
        "bvb": nc.dram_tensor("bvb", [P, H], F32, kind="ExternalInput").ap(),
        "gmb": nc.dram_tensor("gmb", [P, 1], F32, kind="ExternalInput").ap(),
        "idn": nc.dram_tensor("idn", [P, P], F32, kind="ExternalInput").ap(),
        "y": nc.dram_tensor("y", [S, H], F32, kind="ExternalOutput").ap(),
    }
    with tile.TileContext(nc) as tc:
        if reps == 1:
            emit_body(nc, tc, d)
        else:
            with tc.For_i(0, reps, 1):
                emit_body(nc, tc, d)
    nc.compile()
    return nc


_NC = None


def _get_nc():
    global _NC
    if _NC is None:
        _NC = build_program()
    return _NC


def make_in_maps(x, Wq, bq, Wk, bk, Wv, bv, gamma, n_cores=8):
    x = np.asarray(x, np.float32)
    wqk = np.concatenate([np.asarray(Wq, np.float32),
                          np.asarray(Wk, np.float32)], axis=1)
    wqk_b = np.ascontiguousarray(wqk).astype(ml_dtypes.bfloat16)
    gval = np.asarray(gamma, np.float32).reshape(-1)[0]
    # fold gamma into the V projection: softmax(qk^T) @ (gamma*v) + x
    wv_b = np.ascontiguousarray(np.asarray(Wv, np.float32) * gval).astype(
        ml_dtypes.bfloat16)
    bq_c = np.ascontiguousarray(np.asarray(bq, np.float32).reshape(KD, 1))
    bvb = np.ascontiguousarray(
        np.broadcast_to(np.asarray(bv, np.float32) * gval, (P, H)).copy())
    gmb = np.full((P, 1), np.asarray(gamma, np.float32).reshape(-1)[0],
                  np.float32)
    idn = np.eye(P, dtype=np.float32)
    xb = x.astype(ml_dtypes.bfloat16)
    return [
        {"x": np.ascontiguousarray(x[b]), "xb": np.ascontiguousarray(xb[b]),
         "wqk": wqk_b, "wv": wv_b,
         "bq": bq_c, "bvb": bvb, "gmb": gmb, "idn": idn}
        for b in range(n_cores)
    ]


def kernel(x, Wq, bq, Wk, bk, Wv, bv, gamma):
    nc = _get_nc()
    in_maps = make_in_maps(x, Wq, bq, Wk, bk, Wv, bv, gamma)
    res = run_bass_kernel_spmd(nc, in_maps, list(range(8)))
    return np.stack([res.results[c]["y"] for c in range(8)], axis=0)
